# revision 15
# baseline (speedup 1.0000x reference)
"""Trainium2 Bass kernel for nn_BSLoss_13640816132730.

Computes the Black-Scholes PINN loss from reference.py:
  - pde_loss: mean squared PDE residual over the interior of a 4096x4096
    (S x t) grid, with 3-point finite-difference stencils along both axes
    and a clipped second-derivative term.
  - two small boundary losses (far-field BC row, terminal-condition col).

Strategy (8 NeuronCores, data-parallel over t):
  * Each core owns 512 t-columns (+1 halo column each side).
  * The S axis is processed in 33 overlapping 128-row tiles (stride 126), so
    every S-stencil is contained in one tile.
  * Per tile, the heavy lifting runs on the PE array as float32r matmuls with
    banded "stencil" matrices (coefficients folded in host-side):
       PSUM_Z = band_Z^T @ T          (the to-be-clipped V_SS term, scaled)
       PSUM_R = band_NL^T @ T         (linear S-terms + alpha*V)
              + I^T @ T[:, +1]        (t-stencil, right tap)
              + (-I)^T @ T[:, -1]     (t-stencil, left tap)
              + (-I)^T @ clip(PSUM_Z) (subtract the clipped term)
    The clip is a single DVE tensor_scalar (max, min) with per-partition
    bounds; the squared sum is a single ACT Square activation with accum_out.
  * Row interior masking (s=0, s=4095) is via zeroed band columns/bounds plus
    partition-sliced ACT; column masking (t=0, t=4095) and the two boundary
    losses are cheap O(N) host-side numpy corrections.
  * Host gathers per-core [128, 33] partial sums and assembles the scalar.
"""

import math

import numpy as np

import concourse.bacc as bacc
import concourse.bass as bass
import concourse.bass_utils as bass_utils
import concourse.mybir as mybir
import concourse.tile as tile

# ----------------------------------------------------------------------------
# Problem constants (must match reference.py)
# ----------------------------------------------------------------------------
N_S = 4096
N_T = 4096
R = 0.05
SIGMA = 0.2
K_STRIKE = 100.0
T_MAT = 1.0
S_MAX = 300.0
B_BARRIER = 100.0
ALPHA_STR = 30.0
CHI = 6.0
LAMBDA_PDE = 1.0
LAMBDA_BC = 10.0
LAMBDA_TC = 10.0
HUBER_DELTA = 0.01
ALPHA = 2.0 * R / SIGMA**2
TAU_MAX = 0.5 * SIGMA**2 * T_MAT
DU = 1.0 / (N_S - 1)
DT_NORM = TAU_MAX / (N_T - 1)
OMEGA = 2.0 * DT_NORM  # residual is computed on-device scaled by OMEGA

# ----------------------------------------------------------------------------
# Kernel geometry
# ----------------------------------------------------------------------------
N_CORES = 8
OWN_COLS = N_T // N_CORES  # 512 owned t-columns per core
SH_COLS = OWN_COLS + 2  # + halo col each side
TILE_STRIDE = 126  # 126 target rows per 128-row tile
N_TILES = 33  # ceil(4096 / 126)
PAD_ROWS = TILE_STRIDE * (N_TILES - 1) + 128  # 4160

USE_F32R = True  # float32r single-pass matmuls (4x faster than fp32 on PE)


def _solve_depressed_cubic(Q):
    p = CHI
    q = CHI * Q
    sp = math.sqrt(p)
    arg = abs(q) / (2.0 * p * sp / (3.0 * math.sqrt(3.0)))
    arg = max(1.0, arg)
    c = 2.0 * sp * math.cosh(math.acosh(arg) / 3.0)
    return -c if q >= 0 else c


C1 = _solve_depressed_cubic((B_BARRIER - 0.0) / ALPHA_STR)
C2 = _solve_depressed_cubic((B_BARRIER - S_MAX) / ALPHA_STR)


def _stretch_metrics(u):
    """CubicStretching.compute_metrics in float64. u may be any ndarray."""
    L = C2 * u + C1 * (1.0 - u)
    dL = C2 - C1
    S = B_BARRIER + ALPHA_STR * (L**3 / CHI + L)
    dS = ALPHA_STR * dL * (0.5 * L**2 + 1.0)
    d2S = ALPHA_STR * dL * dL * L
    return S, dS, d2S


def _row_coefs(s):
    """Per-row PDE coefficients, s = global S-row index (float64 array)."""
    u = s / (N_S - 1.0)
    S_phys, S_u, S_uu = _stretch_metrics(u)
    S_norm = S_phys / S_MAX
    S_u_n = S_u / S_MAX
    S_uu_n = S_uu / S_MAX
    A = 1.0 / (DU * DU * S_u_n * S_u_n)  # coef on D2 in V_SS
    Bc = -S_uu_n / (2.0 * DU * S_u_n**3)  # coef on D1 in V_SS
    d = S_norm * S_norm  # multiplies clipped V_SS
    g = OMEGA * ALPHA * S_norm / (2.0 * DU * S_u_n)  # coef on D1 (advection)
    return A, Bc, d, g


def _act_slice(p):
    """Valid target partition range [lo, hi) for tile p."""
    lo, hi = 1, 127
    if p == 0:
        lo = 2  # excludes s=0
    if p == N_TILES - 1:
        # s_t = 126*32 + i - 1 = 4031 + i; valid s_t <= 4094 -> i <= 63
        hi = N_S - TILE_STRIDE * (N_TILES - 1)  # 64 -> i in [1, 63]
    return lo, hi


def _jvariant(p):
    """Which of the 3 diagonal mask matrices the t-stencil matmuls use."""
    if p == 0:
        return 0
    if p == N_TILES - 1:
        return 2
    return 1


def _build_tables():
    """Band matrices, clip bounds. Shared by all cores. float32."""
    bandz = np.zeros((N_TILES, 128, 128), np.float32)
    bandnl = np.zeros((N_TILES, 128, 128), np.float32)
    bounds = np.zeros((128, 2 * N_TILES), np.float32)
    for p in range(N_TILES):
        i = np.arange(1, 127)
        s_t = TILE_STRIDE * p + i - 1
        valid = (s_t >= 1) & (s_t <= N_S - 2)
        iv = i[valid]
        A, Bc, d, g = _row_coefs(s_t[valid].astype(np.float64))
        # band_Z: lhsT[src_row, target]; Z = omega*d*(A*D2 + B*D1)
        bandz[p][iv + 1, iv] = OMEGA * d * (A + Bc)
        bandz[p][iv, iv] = OMEGA * d * (-2.0 * A)
        bandz[p][iv - 1, iv] = OMEGA * d * (A - Bc)
        # band_NL: omega*(alpha*V - alpha*S_norm*V_S) -> taps -g, omega*alpha, +g
        bandnl[p][iv + 1, iv] = -g
        bandnl[p][iv, iv] = OMEGA * ALPHA
        bandnl[p][iv - 1, iv] = g
        hi = 100.0 * OMEGA * d
        bounds[iv, 2 * p] = hi
        bounds[iv, 2 * p + 1] = -hi
    # diagonal mask matrices (valid-target selectors) for the t-stencil
    jmask = np.zeros((3, 128, 128), np.float32)
    for var in range(3):
        p = {0: 0, 1: 1, 2: N_TILES - 1}[var]
        lo, hi_i = _act_slice(p)
        idx = np.arange(lo, hi_i)
        jmask[var][idx, idx] = 1.0
    jneg = -jmask
    return bandz, bandnl, bounds, jmask, jneg


def _build_nc():
    dt_mm = mybir.dt.float32r if USE_F32R else mybir.dt.float32
    nc = bacc.Bacc("TRN2", target_bir_lowering=False, debug=False,
                   num_devices=N_CORES)
    vsh = nc.dram_tensor("vsh", [PAD_ROWS, SH_COLS], dt_mm,
                         kind="ExternalInput").ap()
    bandz = nc.dram_tensor("bandz", [N_TILES, 128, 128], dt_mm,
                           kind="ExternalInput").ap()
    bandnl = nc.dram_tensor("bandnl", [N_TILES, 128, 128], dt_mm,
                            kind="ExternalInput").ap()
    jmask_d = nc.dram_tensor("jmask", [3, 128, 128], dt_mm,
                             kind="ExternalInput").ap()
    jneg_d = nc.dram_tensor("jneg", [3, 128, 128], dt_mm,
                            kind="ExternalInput").ap()
    bounds_d = nc.dram_tensor("bounds", [128, 2 * N_TILES], mybir.dt.float32,
                              kind="ExternalInput").ap()
    acc_d = nc.dram_tensor("acc", [128, N_TILES], mybir.dt.float32,
                           kind="ExternalOutput").ap()

    with tile.TileContext(nc) as tc:
        with (
            tc.tile_pool(name="consts", bufs=1) as consts,
            tc.tile_pool(name="inp", bufs=4) as inp,
            tc.tile_pool(name="bands", bufs=4) as bandp,
            tc.tile_pool(name="work", bufs=3) as workp,
            tc.tile_pool(name="psum", bufs=2, space="PSUM") as psump,
        ):
            jmask_t = consts.tile([128, 3, 128], dt_mm)
            jneg_t = consts.tile([128, 3, 128], dt_mm)
            for v in range(3):
                nc.sync.dma_start(out=jmask_t[:, v, :], in_=jmask_d[v])
                nc.sync.dma_start(out=jneg_t[:, v, :], in_=jneg_d[v])
            bounds_t = consts.tile([128, 2 * N_TILES], mybir.dt.float32)
            nc.sync.dma_start(out=bounds_t[:], in_=bounds_d)
            acc_t = consts.tile([128, N_TILES], mybir.dt.float32)
            nc.vector.memset(acc_t[:], 0.0)

            for p in range(N_TILES):
                r0 = TILE_STRIDE * p
                T = inp.tile([128, SH_COLS], dt_mm)
                nc.sync.dma_start(out=T[:], in_=vsh[r0:r0 + 128, :])
                bz = bandp.tile([128, 128], dt_mm)
                nc.sync.dma_start(out=bz[:], in_=bandz[p])
                bn = bandp.tile([128, 128], dt_mm)
                nc.sync.dma_start(out=bn[:], in_=bandnl[p])

                psz = psump.tile([128, OWN_COLS], mybir.dt.float32)
                nc.tensor.matmul(psz[:], lhsT=bz[:], rhs=T[:, 1:1 + OWN_COLS],
                                 start=True, stop=True)

                cl = workp.tile([128, OWN_COLS], dt_mm)
                nc.vector.tensor_scalar(
                    cl[:], psz[:],
                    bounds_t[:, 2 * p + 1:2 * p + 2],  # lo
                    bounds_t[:, 2 * p:2 * p + 1],      # hi
                    mybir.AluOpType.max, mybir.AluOpType.min)

                v = _jvariant(p)
                psr = psump.tile([128, OWN_COLS], mybir.dt.float32)
                nc.tensor.matmul(psr[:], lhsT=bn[:], rhs=T[:, 1:1 + OWN_COLS],
                                 start=True, stop=False)
                nc.tensor.matmul(psr[:], lhsT=jmask_t[:, v, :],
                                 rhs=T[:, 2:2 + OWN_COLS],
                                 start=False, stop=False)
                nc.tensor.matmul(psr[:], lhsT=jneg_t[:, v, :],
                                 rhs=T[:, 0:OWN_COLS],
                                 start=False, stop=False)
                nc.tensor.matmul(psr[:], lhsT=jneg_t[:, 1, :], rhs=cl[:],
                                 start=False, stop=True)

                sq = workp.tile([128, OWN_COLS], mybir.dt.float32)
                nc.scalar.activation(
                    out=sq[:], in_=psr[:],
                    func=mybir.ActivationFunctionType.Square,
                    accum_out=acc_t[:, p:p + 1])

            nc.sync.dma_start(out=acc_d, in_=acc_t[:])
    nc.compile()
    return nc


# ----------------------------------------------------------------------------
# Host-side exact helpers (float64)
# ----------------------------------------------------------------------------
def _host_residual_cols(V, cols):
    """residual[s, j] for s=1..N_S-2 at the given t-columns j (exact formula).

    V is the full [N_S, N_T] float array. Returns [N_S-2, len(cols)]."""
    V = V.astype(np.float64)
    out = []
    s = np.arange(1, N_S - 1)
    A, Bc, d, g = _row_coefs(s.astype(np.float64))
    u = s / (N_S - 1.0)
    S_phys, S_u, S_uu = _stretch_metrics(u)
    S_norm = S_phys / S_MAX
    S_u_n = S_u / S_MAX
    for j in cols:
        jm = max(j - 1, 0)
        jp = min(j + 1, N_T - 1)
        v_c = V[1:-1, j]
        v_up = V[0:-2, j]
        v_dn = V[2:, j]
        D1 = v_dn - v_up
        D2 = v_dn - 2.0 * v_c + v_up
        V_u = D1 / (2.0 * DU)
        V_uu = D2 / (DU * DU)
        V_t = (V[1:-1, jp] - V[1:-1, jm]) / (2.0 * DT_NORM)
        V_S = V_u / S_u_n
        V_SS = (V_uu * S_u_n - V_u * (S_uu / S_MAX)) / S_u_n**3
        V_SS = np.clip(V_SS, -100.0, 100.0)
        res = V_t - S_norm**2 * V_SS - ALPHA * S_norm * V_S + ALPHA * v_c
        out.append(res)
    return np.stack(out, axis=1)


def _host_boundary_losses(V):
    V64 = V.astype(np.float64)
    t = np.linspace(0.0, 1.0, N_T)
    bc_target = 1.0 - K_STRIKE * np.exp(-R * (1.0 - t)) / S_MAX
    loss_smax = np.sum((V64[-1, :] - bc_target) ** 2) / N_T

    u = np.linspace(0.0, 1.0, N_S)
    x = 50.0 * (u - K_STRIKE / S_MAX)
    payoff = np.logaddexp(0.0, x) / 50.0  # softplus(x)/50
    diff = V64[:, -1] - payoff
    ad = np.abs(diff)
    hub = np.where(ad < HUBER_DELTA, 0.5 * diff * diff,
                   HUBER_DELTA * (ad - 0.5 * HUBER_DELTA))
    loss_t = np.sum(hub) / N_S
    return loss_smax, loss_t


def _make_in_maps(V):
    """V: [N_S, N_T] float32. Returns per-core input dicts."""
    bandz, bandnl, bounds, jmask, jneg = _build_tables()
    # pad t with halo columns (edge), pad S rows for the tile grid.
    Vp = np.pad(V, ((0, 0), (1, 1)), mode="edge")  # [4096, 4098]
    vsh_full = np.empty((PAD_ROWS, N_T + 2), np.float32)
    vsh_full[1:N_S + 1] = Vp
    vsh_full[0] = Vp[0]
    vsh_full[N_S + 1:] = Vp[-1]
    in_maps = []
    for k in range(N_CORES):
        c0 = OWN_COLS * k
        shard = np.ascontiguousarray(vsh_full[:, c0:c0 + SH_COLS])
        in_maps.append({
            "vsh": shard,
            "bandz": bandz,
            "bandnl": bandnl,
            "jmask": jmask,
            "jneg": jneg,
            "bounds": bounds,
        })
    return in_maps


TRACE = False
LAST_RESULTS = None


def _assemble(V, accs):
    """Combine device partial sums + host boundary terms into the loss."""
    device_r2 = float(sum(a.astype(np.float64).sum() for a in accs))
    # device_r2 = omega^2 * sum residual^2 over s in [1,4094], j in [0,4095]
    corr = _host_residual_cols(V, [0, N_T - 1])
    corr_sum = float((corr ** 2).sum())
    pde_sum = device_r2 / (OMEGA * OMEGA) - corr_sum
    n_int = (N_S - 2) * (N_T - 2)
    pde_loss = pde_sum / n_int
    loss_smax, loss_t = _host_boundary_losses(V)
    total = (LAMBDA_PDE * pde_loss + LAMBDA_BC * loss_smax
             + LAMBDA_TC * loss_t)
    return np.float32(total)


def kernel(V_norm):
    global LAST_RESULTS
    V = np.ascontiguousarray(np.asarray(V_norm, dtype=np.float32)[0, 0])
    in_maps = _make_in_maps(V)
    nc = _build_nc()
    res = bass_utils.run_bass_kernel_spmd(
        nc, in_maps, core_ids=list(range(N_CORES)), trace=TRACE)
    LAST_RESULTS = res
    accs = [r["acc"] for r in res.results]
    return _assemble(V, accs)


# revision 20
# speedup vs baseline: 1.2163x; 1.2163x over previous
"""Trainium2 Bass kernel for nn_BSLoss_13640816132730.

Computes the Black-Scholes PINN loss from reference.py:
  - pde_loss: mean squared PDE residual over the interior of a 4096x4096
    (S x t) grid, with 3-point finite-difference stencils along both axes
    and a clipped second-derivative term.
  - two small boundary losses (far-field BC row, terminal-condition col).

Strategy (8 NeuronCores, data-parallel over t):
  * Each core owns 512 t-columns (+1 halo column each side).
  * The S axis is processed in 33 overlapping 128-row tiles (stride 126), so
    every S-stencil is contained in one tile.
  * Per tile, the heavy lifting runs on the PE array as float32r matmuls with
    banded "stencil" matrices (coefficients folded in host-side):
       PSUM_Z = band_Z^T @ T          (the to-be-clipped V_SS term, scaled)
       PSUM_R = band_NL^T @ T         (linear S-terms + alpha*V)
              + I^T @ T[:, +1]        (t-stencil, right tap)
              + (-I)^T @ T[:, -1]     (t-stencil, left tap)
              + (-I)^T @ clip(PSUM_Z) (subtract the clipped term)
    The clip is a single DVE tensor_scalar (max, min) with per-partition
    bounds; the squared sum is a single ACT Square activation with accum_out.
  * Row interior masking (s=0, s=4095) is via zeroed band columns/bounds plus
    partition-sliced ACT; column masking (t=0, t=4095) and the two boundary
    losses are cheap O(N) host-side numpy corrections.
  * Host gathers per-core [128, 33] partial sums and assembles the scalar.
"""

import math

import ml_dtypes
import numpy as np

import concourse.bacc as bacc
import concourse.bass as bass
import concourse.bass_utils as bass_utils
import concourse.mybir as mybir
import concourse.tile as tile

# ----------------------------------------------------------------------------
# Problem constants (must match reference.py)
# ----------------------------------------------------------------------------
N_S = 4096
N_T = 4096
R = 0.05
SIGMA = 0.2
K_STRIKE = 100.0
T_MAT = 1.0
S_MAX = 300.0
B_BARRIER = 100.0
ALPHA_STR = 30.0
CHI = 6.0
LAMBDA_PDE = 1.0
LAMBDA_BC = 10.0
LAMBDA_TC = 10.0
HUBER_DELTA = 0.01
ALPHA = 2.0 * R / SIGMA**2
TAU_MAX = 0.5 * SIGMA**2 * T_MAT
DU = 1.0 / (N_S - 1)
DT_NORM = TAU_MAX / (N_T - 1)
OMEGA = 2.0 * DT_NORM  # residual is computed on-device scaled by OMEGA

# ----------------------------------------------------------------------------
# Kernel geometry
# ----------------------------------------------------------------------------
N_CORES = 8
OWN_COLS = N_T // N_CORES  # 512 owned t-columns per core
SH_COLS = OWN_COLS + 2  # + halo col each side
TILE_STRIDE = 126  # 126 target rows per 128-row tile
N_TILES = 33  # ceil(4096 / 126)
PAD_ROWS = TILE_STRIDE * (N_TILES - 1) + 128  # 4160

USE_F32R = True  # float32r single-pass matmuls (4x faster than fp32 on PE)
USE_BF16 = True  # bf16 data/matmuls: ~2x faster PE + half the DMA traffic.
# The loss is a 16.7M-element mean of squares, so per-element bf16
# quantization noise averages out (measured ~1e-5 rel vs fp32).


def _solve_depressed_cubic(Q):
    p = CHI
    q = CHI * Q
    sp = math.sqrt(p)
    arg = abs(q) / (2.0 * p * sp / (3.0 * math.sqrt(3.0)))
    arg = max(1.0, arg)
    c = 2.0 * sp * math.cosh(math.acosh(arg) / 3.0)
    return -c if q >= 0 else c


C1 = _solve_depressed_cubic((B_BARRIER - 0.0) / ALPHA_STR)
C2 = _solve_depressed_cubic((B_BARRIER - S_MAX) / ALPHA_STR)


def _stretch_metrics(u):
    """CubicStretching.compute_metrics in float64. u may be any ndarray."""
    L = C2 * u + C1 * (1.0 - u)
    dL = C2 - C1
    S = B_BARRIER + ALPHA_STR * (L**3 / CHI + L)
    dS = ALPHA_STR * dL * (0.5 * L**2 + 1.0)
    d2S = ALPHA_STR * dL * dL * L
    return S, dS, d2S


def _row_coefs(s):
    """Per-row PDE coefficients, s = global S-row index (float64 array)."""
    u = s / (N_S - 1.0)
    S_phys, S_u, S_uu = _stretch_metrics(u)
    S_norm = S_phys / S_MAX
    S_u_n = S_u / S_MAX
    S_uu_n = S_uu / S_MAX
    A = 1.0 / (DU * DU * S_u_n * S_u_n)  # coef on D2 in V_SS
    Bc = -S_uu_n / (2.0 * DU * S_u_n**3)  # coef on D1 in V_SS
    d = S_norm * S_norm  # multiplies clipped V_SS
    g = OMEGA * ALPHA * S_norm / (2.0 * DU * S_u_n)  # coef on D1 (advection)
    return A, Bc, d, g


def _act_slice(p):
    """Valid target partition range [lo, hi) for tile p."""
    lo, hi = 1, 127
    if p == 0:
        lo = 2  # excludes s=0
    if p == N_TILES - 1:
        # s_t = 126*32 + i - 1 = 4031 + i; valid s_t <= 4094 -> i <= 63
        hi = N_S - TILE_STRIDE * (N_TILES - 1)  # 64 -> i in [1, 63]
    return lo, hi


def _jvariant(p):
    """Which of the 3 diagonal mask matrices the t-stencil matmuls use."""
    if p == 0:
        return 0
    if p == N_TILES - 1:
        return 2
    return 1


def _build_tables():
    """Band matrices, clip bounds. Shared by all cores. float32."""
    bandz = np.zeros((N_TILES, 128, 128), np.float32)
    bandnl = np.zeros((N_TILES, 128, 128), np.float32)
    bounds = np.zeros((128, 2 * N_TILES), np.float32)
    for p in range(N_TILES):
        i = np.arange(1, 127)
        s_t = TILE_STRIDE * p + i - 1
        valid = (s_t >= 1) & (s_t <= N_S - 2)
        iv = i[valid]
        A, Bc, d, g = _row_coefs(s_t[valid].astype(np.float64))
        # band_Z: lhsT[src_row, target]; Z = omega*d*(A*D2 + B*D1)
        bandz[p][iv + 1, iv] = OMEGA * d * (A + Bc)
        bandz[p][iv, iv] = OMEGA * d * (-2.0 * A)
        bandz[p][iv - 1, iv] = OMEGA * d * (A - Bc)
        # band_NL: omega*(alpha*V - alpha*S_norm*V_S) -> taps -g, omega*alpha, +g
        bandnl[p][iv + 1, iv] = -g
        bandnl[p][iv, iv] = OMEGA * ALPHA
        bandnl[p][iv - 1, iv] = g
        hi = 100.0 * OMEGA * d
        bounds[iv, 2 * p] = hi
        bounds[iv, 2 * p + 1] = -hi
    # diagonal mask matrices (valid-target selectors) for the t-stencil
    jmask = np.zeros((3, 128, 128), np.float32)
    for var in range(3):
        p = {0: 0, 1: 1, 2: N_TILES - 1}[var]
        lo, hi_i = _act_slice(p)
        idx = np.arange(lo, hi_i)
        jmask[var][idx, idx] = 1.0
    jneg = -jmask[1]
    bands = np.concatenate([bandz, bandnl], axis=2)  # [33, 128, 256]
    return bands, bounds, jmask, jneg


def _dt_mm():
    if USE_BF16:
        return mybir.dt.bfloat16
    return mybir.dt.float32r if USE_F32R else mybir.dt.float32


def _np_mm():
    return ml_dtypes.bfloat16 if USE_BF16 else np.float32


def _build_nc():
    dt_mm = _dt_mm()
    nc = bacc.Bacc("TRN2", target_bir_lowering=False, debug=False,
                   num_devices=N_CORES)
    vsh = nc.dram_tensor("vsh", [PAD_ROWS, SH_COLS], dt_mm,
                         kind="ExternalInput").ap()
    # bands[p][:, 0:128] = band_Z, [:, 128:256] = band_NL
    bands_d = nc.dram_tensor("bands", [N_TILES, 128, 256], dt_mm,
                             kind="ExternalInput").ap()
    jmask_d = nc.dram_tensor("jmask", [3, 128, 128], dt_mm,
                             kind="ExternalInput").ap()
    jneg_d = nc.dram_tensor("jneg", [128, 128], dt_mm,
                            kind="ExternalInput").ap()
    bounds_d = nc.dram_tensor("bounds", [128, 2 * N_TILES], mybir.dt.float32,
                              kind="ExternalInput").ap()
    acc_d = nc.dram_tensor("acc", [128, N_TILES], mybir.dt.float32,
                           kind="ExternalOutput").ap()

    with tile.TileContext(nc) as tc:
        with (
            tc.tile_pool(name="consts", bufs=1) as consts,
            tc.tile_pool(name="inp", bufs=4) as inp,
            tc.tile_pool(name="bands", bufs=4) as bandp,
            tc.tile_pool(name="work", bufs=3) as workp,
            tc.tile_pool(name="psum", bufs=2, space="PSUM") as psump,
        ):
            jmask_t = consts.tile([128, 3, 128], dt_mm)
            for v in range(3):
                nc.sync.dma_start(out=jmask_t[:, v, :], in_=jmask_d[v])
            jneg_t = consts.tile([128, 128], dt_mm)
            nc.sync.dma_start(out=jneg_t[:], in_=jneg_d)
            bounds_t = consts.tile([128, 2 * N_TILES], mybir.dt.float32)
            nc.sync.dma_start(out=bounds_t[:], in_=bounds_d)
            acc_t = consts.tile([128, N_TILES], mybir.dt.float32)
            nc.vector.memset(acc_t[:], 0.0)

            for p in range(N_TILES):
                r0 = TILE_STRIDE * p
                T = inp.tile([128, SH_COLS], dt_mm)
                nc.sync.dma_start(out=T[:], in_=vsh[r0:r0 + 128, :])
                bd = bandp.tile([128, 256], dt_mm)
                nc.sync.dma_start(out=bd[:], in_=bands_d[p])

                psz = psump.tile([128, OWN_COLS], mybir.dt.float32)
                nc.tensor.matmul(psz[:], lhsT=bd[:, 0:128],
                                 rhs=T[:, 1:1 + OWN_COLS],
                                 start=True, stop=True)

                cl = workp.tile([128, OWN_COLS], dt_mm)
                nc.vector.tensor_scalar(
                    cl[:], psz[:],
                    bounds_t[:, 2 * p + 1:2 * p + 2],  # lo
                    bounds_t[:, 2 * p:2 * p + 1],      # hi
                    mybir.AluOpType.max, mybir.AluOpType.min)

                # t-stencil difference (J-masking happens in the matmul)
                W = workp.tile([128, OWN_COLS], dt_mm)
                nc.vector.tensor_sub(W[:], T[:, 2:2 + OWN_COLS],
                                     T[:, 0:OWN_COLS])

                v = _jvariant(p)
                psr = psump.tile([128, OWN_COLS], mybir.dt.float32)
                nc.tensor.matmul(psr[:], lhsT=bd[:, 128:256],
                                 rhs=T[:, 1:1 + OWN_COLS],
                                 start=True, stop=False)
                nc.tensor.matmul(psr[:], lhsT=jmask_t[:, v, :], rhs=W[:],
                                 start=False, stop=False)
                nc.tensor.matmul(psr[:], lhsT=jneg_t[:], rhs=cl[:],
                                 start=False, stop=True)

                sq = workp.tile([128, OWN_COLS], mybir.dt.float32)
                nc.scalar.activation(
                    out=sq[:], in_=psr[:],
                    func=mybir.ActivationFunctionType.Square,
                    accum_out=acc_t[:, p:p + 1])

            nc.sync.dma_start(out=acc_d, in_=acc_t[:])
    nc.compile()
    return nc


# ----------------------------------------------------------------------------
# Host-side exact helpers (float64)
# ----------------------------------------------------------------------------
def _host_residual_cols(V, cols):
    """residual[s, j] for s=1..N_S-2 at the given t-columns j (exact formula).

    V is the full [N_S, N_T] float array. Returns [N_S-2, len(cols)]."""
    V = V.astype(np.float64)
    out = []
    s = np.arange(1, N_S - 1)
    A, Bc, d, g = _row_coefs(s.astype(np.float64))
    u = s / (N_S - 1.0)
    S_phys, S_u, S_uu = _stretch_metrics(u)
    S_norm = S_phys / S_MAX
    S_u_n = S_u / S_MAX
    for j in cols:
        jm = max(j - 1, 0)
        jp = min(j + 1, N_T - 1)
        v_c = V[1:-1, j]
        v_up = V[0:-2, j]
        v_dn = V[2:, j]
        D1 = v_dn - v_up
        D2 = v_dn - 2.0 * v_c + v_up
        V_u = D1 / (2.0 * DU)
        V_uu = D2 / (DU * DU)
        V_t = (V[1:-1, jp] - V[1:-1, jm]) / (2.0 * DT_NORM)
        V_S = V_u / S_u_n
        V_SS = (V_uu * S_u_n - V_u * (S_uu / S_MAX)) / S_u_n**3
        V_SS = np.clip(V_SS, -100.0, 100.0)
        res = V_t - S_norm**2 * V_SS - ALPHA * S_norm * V_S + ALPHA * v_c
        out.append(res)
    return np.stack(out, axis=1)


def _host_boundary_losses(V):
    V64 = V.astype(np.float64)
    t = np.linspace(0.0, 1.0, N_T)
    bc_target = 1.0 - K_STRIKE * np.exp(-R * (1.0 - t)) / S_MAX
    loss_smax = np.sum((V64[-1, :] - bc_target) ** 2) / N_T

    u = np.linspace(0.0, 1.0, N_S)
    x = 50.0 * (u - K_STRIKE / S_MAX)
    payoff = np.logaddexp(0.0, x) / 50.0  # softplus(x)/50
    diff = V64[:, -1] - payoff
    ad = np.abs(diff)
    hub = np.where(ad < HUBER_DELTA, 0.5 * diff * diff,
                   HUBER_DELTA * (ad - 0.5 * HUBER_DELTA))
    loss_t = np.sum(hub) / N_S
    return loss_smax, loss_t


def _make_in_maps(V):
    """V: [N_S, N_T] float32. Returns per-core input dicts."""
    bands, bounds, jmask, jneg = _build_tables()
    np_mm = _np_mm()
    bands = bands.astype(np_mm)
    jmask = jmask.astype(np_mm)
    jneg = jneg.astype(np_mm)
    # pad t with halo columns (edge), pad S rows for the tile grid.
    Vp = np.pad(V, ((0, 0), (1, 1)), mode="edge")  # [4096, 4098]
    vsh_full = np.empty((PAD_ROWS, N_T + 2), np.float32)
    vsh_full[1:N_S + 1] = Vp
    vsh_full[0] = Vp[0]
    vsh_full[N_S + 1:] = Vp[-1]
    vsh_full = vsh_full.astype(np_mm)
    in_maps = []
    for k in range(N_CORES):
        c0 = OWN_COLS * k
        shard = np.ascontiguousarray(vsh_full[:, c0:c0 + SH_COLS])
        in_maps.append({
            "vsh": shard,
            "bands": bands,
            "jmask": jmask,
            "jneg": jneg,
            "bounds": bounds,
        })
    return in_maps


TRACE = False
LAST_RESULTS = None


def _assemble(V, accs):
    """Combine device partial sums + host boundary terms into the loss."""
    device_r2 = float(sum(a.astype(np.float64).sum() for a in accs))
    # device_r2 = omega^2 * sum residual^2 over s in [1,4094], j in [0,4095]
    corr = _host_residual_cols(V, [0, N_T - 1])
    corr_sum = float((corr ** 2).sum())
    pde_sum = device_r2 / (OMEGA * OMEGA) - corr_sum
    n_int = (N_S - 2) * (N_T - 2)
    pde_loss = pde_sum / n_int
    loss_smax, loss_t = _host_boundary_losses(V)
    total = (LAMBDA_PDE * pde_loss + LAMBDA_BC * loss_smax
             + LAMBDA_TC * loss_t)
    return np.float32(total)


def kernel(V_norm):
    global LAST_RESULTS
    V = np.ascontiguousarray(np.asarray(V_norm, dtype=np.float32)[0, 0])
    in_maps = _make_in_maps(V)
    nc = _build_nc()
    res = bass_utils.run_bass_kernel_spmd(
        nc, in_maps, core_ids=list(range(N_CORES)), trace=TRACE)
    LAST_RESULTS = res
    accs = [r["acc"] for r in res.results]
    return _assemble(V, accs)


# revision 22
# speedup vs baseline: 1.2367x; 1.0167x over previous
"""Trainium2 Bass kernel for nn_BSLoss_13640816132730.

Computes the Black-Scholes PINN loss from reference.py:
  - pde_loss: mean squared PDE residual over the interior of a 4096x4096
    (S x t) grid, with 3-point finite-difference stencils along both axes
    and a clipped second-derivative term.
  - two small boundary losses (far-field BC row, terminal-condition col).

Strategy (8 NeuronCores, data-parallel over t):
  * Each core owns 512 t-columns (+1 halo column each side).
  * The S axis is processed in 33 overlapping 128-row tiles (stride 126), so
    every S-stencil is contained in one tile; tiles are handled in PAIRS so
    DMA / DVE / ACT fixed costs amortize over 1024-column operations.
  * Per tile the heavy lifting runs on the PE array as bf16 matmuls with
    banded "stencil" matrices (coefficients folded in host-side):
       PSUM_Z = band_Z^T @ T              (to-be-clipped V_SS term, scaled
                                           so the clip bounds are +-1)
       cl     = clip(PSUM_Z, -1, 1)       (one DVE tensor_scalar per pair)
       W      = T[:,+1] - T[:,-1]         (t-stencil; DVE/GPSIMD alternating)
       PSUM_R = band_NL^T @ T             (linear S-terms + alpha*V)
              + J^T @ W                   (diagonal row-validity mask)
              + band_SD^T @ cl            (-100*omega*d(s) diagonal)
       acc   += sum(Square(PSUM_R))       (one ACT activation per pair)
  * All inputs for a pair (2x V tile + 6 band matrices) ship as ONE DMA.
  * Row interior masking (s=0, s=4095, overlap rows) is via zeroed band
    columns; column masking (t=0, t=4095) and the two boundary losses are
    cheap O(N) host-side numpy corrections.
  * Host gathers per-core [128, 17] partial sums and assembles the scalar.

bf16 note: the loss is a 16.7M-element mean of squares, so per-element bf16
quantization noise averages out (measured ~1e-5 relative vs the fp32
reference; fp32/f32r variant of this kernel measured 7.6e-8).
"""

import math

import ml_dtypes
import numpy as np

import concourse.bacc as bacc
import concourse.bass as bass  # noqa: F401  (kept for users of this module)
import concourse.bass_utils as bass_utils
import concourse.mybir as mybir
import concourse.tile as tile

# ----------------------------------------------------------------------------
# Problem constants (must match reference.py)
# ----------------------------------------------------------------------------
N_S = 4096
N_T = 4096
R = 0.05
SIGMA = 0.2
K_STRIKE = 100.0
T_MAT = 1.0
S_MAX = 300.0
B_BARRIER = 100.0
ALPHA_STR = 30.0
CHI = 6.0
LAMBDA_PDE = 1.0
LAMBDA_BC = 10.0
LAMBDA_TC = 10.0
HUBER_DELTA = 0.01
ALPHA = 2.0 * R / SIGMA**2
TAU_MAX = 0.5 * SIGMA**2 * T_MAT
DU = 1.0 / (N_S - 1)
DT_NORM = TAU_MAX / (N_T - 1)
OMEGA = 2.0 * DT_NORM  # residual is computed on-device scaled by OMEGA

# ----------------------------------------------------------------------------
# Kernel geometry
# ----------------------------------------------------------------------------
N_CORES = 8
OWN_COLS = N_T // N_CORES  # 512 owned t-columns per core
SH_COLS = OWN_COLS + 2  # + halo col each side
TILE_STRIDE = 126  # 126 target rows per 128-row tile
N_TILES = 33  # ceil(4096 / 126)
N_PAIRS = (N_TILES + 1) // 2  # 17 (last pair has a zero-padded dummy slot)
PAD_ROWS = TILE_STRIDE * (N_TILES - 1) + 128  # 4160

# megatile layout (bf16 elements per partition-row, per tile-pair):
#   [T_a 514][T_b 514][bz_a 128][bn_a 128][bs_a 128][bz_b 128][bn_b 128][bs_b 128]
MEG_T0 = 0
MEG_T1 = SH_COLS
MEG_B0 = 2 * SH_COLS  # 1028
MEG_COLS = 2 * SH_COLS + 6 * 128  # 1796


def _solve_depressed_cubic(Q):
    p = CHI
    q = CHI * Q
    sp = math.sqrt(p)
    arg = abs(q) / (2.0 * p * sp / (3.0 * math.sqrt(3.0)))
    arg = max(1.0, arg)
    c = 2.0 * sp * math.cosh(math.acosh(arg) / 3.0)
    return -c if q >= 0 else c


C1 = _solve_depressed_cubic((B_BARRIER - 0.0) / ALPHA_STR)
C2 = _solve_depressed_cubic((B_BARRIER - S_MAX) / ALPHA_STR)


def _stretch_metrics(u):
    """CubicStretching.compute_metrics in float64. u may be any ndarray."""
    L = C2 * u + C1 * (1.0 - u)
    dL = C2 - C1
    S = B_BARRIER + ALPHA_STR * (L**3 / CHI + L)
    dS = ALPHA_STR * dL * (0.5 * L**2 + 1.0)
    d2S = ALPHA_STR * dL * dL * L
    return S, dS, d2S


def _row_coefs(s):
    """Per-row PDE coefficients, s = global S-row index (float64 array)."""
    u = s / (N_S - 1.0)
    S_phys, S_u, S_uu = _stretch_metrics(u)
    S_norm = S_phys / S_MAX
    S_u_n = S_u / S_MAX
    S_uu_n = S_uu / S_MAX
    A = 1.0 / (DU * DU * S_u_n * S_u_n)  # coef on D2 in V_SS
    Bc = -S_uu_n / (2.0 * DU * S_u_n**3)  # coef on D1 in V_SS
    d = S_norm * S_norm  # multiplies clipped V_SS
    g = OMEGA * ALPHA * S_norm / (2.0 * DU * S_u_n)  # coef on D1 (advection)
    return A, Bc, d, g


def _valid_targets(p):
    """Valid target partitions i (1..126) for tile p and their s-rows."""
    i = np.arange(1, 127)
    s_t = TILE_STRIDE * p + i - 1
    valid = (s_t >= 1) & (s_t <= N_S - 2)
    return i[valid], s_t[valid]


def _jvariant(p):
    """Which of the diagonal mask matrices the t-stencil matmul uses."""
    if p == 0:
        return 0
    if p == N_TILES - 1:
        return 2
    return 1


def _build_tables():
    """Per-tile band matrices [33, 128, 384] and J-mask matrices [4,128,128].

    bands[p][:, 0:128]   = band_Z (clip term, scaled so bounds are +-1)
    bands[p][:, 128:256] = band_NL (linear S terms)
    bands[p][:, 256:384] = band_SD (diagonal -100*omega*d)
    """
    bands = np.zeros((N_TILES, 128, 384), np.float64)
    for p in range(N_TILES):
        iv, s_t = _valid_targets(p)
        A, Bc, d, g = _row_coefs(s_t.astype(np.float64))
        bz = bands[p][:, 0:128]
        bn = bands[p][:, 128:256]
        bs = bands[p][:, 256:384]
        # band_Z': Z' = (A*D2 + B*D1)/100 so that clip bounds are +-1
        bz[iv + 1, iv] = (A + Bc) / 100.0
        bz[iv, iv] = -2.0 * A / 100.0
        bz[iv - 1, iv] = (A - Bc) / 100.0
        # band_NL: omega*(alpha*V - alpha*S_norm*V_S) -> taps -g, w*alpha, +g
        bn[iv + 1, iv] = -g
        bn[iv, iv] = OMEGA * ALPHA
        bn[iv - 1, iv] = g
        # subtract the clipped term scaled back by 100*omega*d(s)
        bs[iv, iv] = -100.0 * OMEGA * d
    jmask = np.zeros((4, 128, 128), np.float64)
    for var, p in {0: 0, 1: 1, 2: N_TILES - 1}.items():
        iv, _ = _valid_targets(p)
        jmask[var][iv, iv] = 1.0
    # variant 3 stays all-zero (dummy slot of the last pair)
    return bands, jmask


def _build_nc():
    dt_mm = mybir.dt.bfloat16
    f32 = mybir.dt.float32
    nc = bacc.Bacc("TRN2", target_bir_lowering=False, debug=False,
                   num_devices=N_CORES)
    meg_d = nc.dram_tensor("meg", [N_PAIRS, 128, MEG_COLS], dt_mm,
                           kind="ExternalInput").ap()
    jmask_d = nc.dram_tensor("jmask", [4, 128, 128], dt_mm,
                             kind="ExternalInput").ap()
    acc_d = nc.dram_tensor("acc", [128, N_PAIRS], f32,
                           kind="ExternalOutput").ap()

    with tile.TileContext(nc) as tc:
        with (
            tc.tile_pool(name="consts", bufs=1) as consts,
            tc.tile_pool(name="inp", bufs=3) as inp,
            tc.tile_pool(name="work", bufs=3) as workp,
            tc.tile_pool(name="psum", bufs=2, space="PSUM") as psump,
        ):
            jmask_t = consts.tile([128, 4, 128], dt_mm)
            for v in range(4):
                nc.sync.dma_start(out=jmask_t[:, v, :], in_=jmask_d[v])
            acc_t = consts.tile([128, N_PAIRS], f32)
            nc.vector.memset(acc_t[:], 0.0)

            for k in range(N_PAIRS):
                M = inp.tile([128, MEG_COLS], dt_mm)
                nc.sync.dma_start(out=M[:], in_=meg_d[k])

                psz = psump.tile([128, 2 * OWN_COLS], f32)
                psr = psump.tile([128, 2 * OWN_COLS], f32)
                for h in range(2):  # the two tiles of the pair
                    b0 = MEG_B0 + h * 384
                    t0 = h * SH_COLS
                    nc.tensor.matmul(
                        psz[:, h * OWN_COLS:(h + 1) * OWN_COLS],
                        lhsT=M[:, b0:b0 + 128],
                        rhs=M[:, t0 + 1:t0 + 1 + OWN_COLS],
                        start=True, stop=True)

                cl = workp.tile([128, 2 * OWN_COLS], dt_mm)
                nc.vector.tensor_scalar(
                    cl[:], psz[:], -1.0, 1.0,
                    mybir.AluOpType.max, mybir.AluOpType.min)

                # t-stencil difference, both tiles at once; alternate the
                # engine so DVE (busy with clips) and GPSIMD share the load.
                W = workp.tile([128, 2, OWN_COLS], dt_mm)
                tpair = M[:, 0:2 * SH_COLS].rearrange(
                    "p (a c) -> p a c", a=2)
                eng = nc.vector if (k % 2 == 0) else nc.gpsimd
                eng.tensor_sub(W[:], tpair[:, :, 2:2 + OWN_COLS],
                               tpair[:, :, 0:OWN_COLS])

                for h in range(2):
                    p = 2 * k + h
                    b0 = MEG_B0 + h * 384
                    t0 = h * SH_COLS
                    v = _jvariant(p) if p < N_TILES else 3
                    out_h = psr[:, h * OWN_COLS:(h + 1) * OWN_COLS]
                    nc.tensor.matmul(
                        out_h, lhsT=M[:, b0 + 128:b0 + 256],
                        rhs=M[:, t0 + 1:t0 + 1 + OWN_COLS],
                        start=True, stop=False)
                    nc.tensor.matmul(
                        out_h, lhsT=jmask_t[:, v, :], rhs=W[:, h, :],
                        start=False, stop=False)
                    nc.tensor.matmul(
                        out_h, lhsT=M[:, b0 + 256:b0 + 384],
                        rhs=cl[:, h * OWN_COLS:(h + 1) * OWN_COLS],
                        start=False, stop=True)

                sq = workp.tile([128, 2 * OWN_COLS], f32)
                nc.scalar.activation(
                    out=sq[:], in_=psr[:],
                    func=mybir.ActivationFunctionType.Square,
                    accum_out=acc_t[:, k:k + 1])

            nc.sync.dma_start(out=acc_d, in_=acc_t[:])
    nc.compile()
    return nc


# ----------------------------------------------------------------------------
# Host-side exact helpers (float64)
# ----------------------------------------------------------------------------
def _host_residual_cols(V, cols):
    """residual[s, j] for s=1..N_S-2 at the given t-columns j (exact formula).

    V is the full [N_S, N_T] float array. Returns [N_S-2, len(cols)]."""
    V = V.astype(np.float64)
    out = []
    s = np.arange(1, N_S - 1)
    u = s / (N_S - 1.0)
    S_phys, S_u, S_uu = _stretch_metrics(u)
    S_norm = S_phys / S_MAX
    S_u_n = S_u / S_MAX
    for j in cols:
        jm = max(j - 1, 0)
        jp = min(j + 1, N_T - 1)
        v_c = V[1:-1, j]
        v_up = V[0:-2, j]
        v_dn = V[2:, j]
        D1 = v_dn - v_up
        D2 = v_dn - 2.0 * v_c + v_up
        V_u = D1 / (2.0 * DU)
        V_uu = D2 / (DU * DU)
        V_t = (V[1:-1, jp] - V[1:-1, jm]) / (2.0 * DT_NORM)
        V_S = V_u / S_u_n
        V_SS = (V_uu * S_u_n - V_u * (S_uu / S_MAX)) / S_u_n**3
        V_SS = np.clip(V_SS, -100.0, 100.0)
        res = V_t - S_norm**2 * V_SS - ALPHA * S_norm * V_S + ALPHA * v_c
        out.append(res)
    return np.stack(out, axis=1)


def _host_boundary_losses(V):
    V64 = V.astype(np.float64)
    t = np.linspace(0.0, 1.0, N_T)
    bc_target = 1.0 - K_STRIKE * np.exp(-R * (1.0 - t)) / S_MAX
    loss_smax = np.sum((V64[-1, :] - bc_target) ** 2) / N_T

    u = np.linspace(0.0, 1.0, N_S)
    x = 50.0 * (u - K_STRIKE / S_MAX)
    payoff = np.logaddexp(0.0, x) / 50.0  # softplus(x)/50
    diff = V64[:, -1] - payoff
    ad = np.abs(diff)
    hub = np.where(ad < HUBER_DELTA, 0.5 * diff * diff,
                   HUBER_DELTA * (ad - 0.5 * HUBER_DELTA))
    loss_t = np.sum(hub) / N_S
    return loss_smax, loss_t


def _make_in_maps(V):
    """V: [N_S, N_T] float32. Returns per-core input dicts."""
    bands, jmask = _build_tables()
    bands16 = bands.astype(ml_dtypes.bfloat16)
    jmask16 = jmask.astype(ml_dtypes.bfloat16)
    # pad t with halo columns (edge), pad S rows for the tile grid.
    Vp = np.pad(V, ((0, 0), (1, 1)), mode="edge")  # [4096, 4098]
    vsh_full = np.empty((PAD_ROWS, N_T + 2), np.float32)
    vsh_full[1:N_S + 1] = Vp
    vsh_full[0] = Vp[0]
    vsh_full[N_S + 1:] = Vp[-1]
    vsh16 = vsh_full.astype(ml_dtypes.bfloat16)

    in_maps = []
    for k in range(N_CORES):
        c0 = OWN_COLS * k
        meg = np.zeros((N_PAIRS, 128, MEG_COLS), ml_dtypes.bfloat16)
        for pair in range(N_PAIRS):
            for h in range(2):
                p = 2 * pair + h
                if p >= N_TILES:
                    continue  # dummy slot stays zero
                r0 = TILE_STRIDE * p
                meg[pair, :, h * SH_COLS:(h + 1) * SH_COLS] = \
                    vsh16[r0:r0 + 128, c0:c0 + SH_COLS]
                meg[pair, :, MEG_B0 + h * 384:MEG_B0 + (h + 1) * 384] = \
                    bands16[p]
        in_maps.append({"meg": meg, "jmask": jmask16})
    return in_maps


TRACE = False
LAST_RESULTS = None


def _assemble(V, accs):
    """Combine device partial sums + host boundary terms into the loss."""
    device_r2 = float(sum(a.astype(np.float64).sum() for a in accs))
    # device_r2 = omega^2 * sum residual^2 over s in [1,4094], j in [0,4095]
    corr = _host_residual_cols(V, [0, N_T - 1])
    corr_sum = float((corr ** 2).sum())
    pde_sum = device_r2 / (OMEGA * OMEGA) - corr_sum
    n_int = (N_S - 2) * (N_T - 2)
    pde_loss = pde_sum / n_int
    loss_smax, loss_t = _host_boundary_losses(V)
    total = (LAMBDA_PDE * pde_loss + LAMBDA_BC * loss_smax
             + LAMBDA_TC * loss_t)
    return np.float32(total)


def kernel(V_norm):
    global LAST_RESULTS
    V = np.ascontiguousarray(np.asarray(V_norm, dtype=np.float32)[0, 0])
    in_maps = _make_in_maps(V)
    nc = _build_nc()
    res = bass_utils.run_bass_kernel_spmd(
        nc, in_maps, core_ids=list(range(N_CORES)), trace=TRACE)
    LAST_RESULTS = res
    accs = [r["acc"] for r in res.results]
    return _assemble(V, accs)


# revision 24
# speedup vs baseline: 1.4042x; 1.1355x over previous
"""Trainium2 Bass kernel for nn_BSLoss_13640816132730.

Computes the Black-Scholes PINN loss from reference.py:
  - pde_loss: mean squared PDE residual over the interior of a 4096x4096
    (S x t) grid, with 3-point finite-difference stencils along both axes
    and a clipped second-derivative term.
  - two small boundary losses (far-field BC row, terminal-condition col).

Strategy (8 NeuronCores, data-parallel over t):
  * Each core owns 512 t-columns (+1 halo column each side).
  * The S axis is processed in 33 overlapping 128-row tiles (stride 126), so
    every S-stencil is contained in one tile; tiles are handled in PAIRS so
    DMA / DVE / ACT fixed costs amortize over 1024-column operations.
  * Per tile the heavy lifting runs on the PE array as bf16 matmuls with
    banded "stencil" matrices (coefficients folded in host-side):
       PSUM_Z = band_Z^T @ T              (to-be-clipped V_SS term, scaled
                                           so the clip bounds are +-1)
       cl     = clip(PSUM_Z, -1, 1)       (one DVE tensor_scalar per pair)
       W      = T[:,+1] - T[:,-1]         (t-stencil; DVE/GPSIMD alternating)
       PSUM_R = band_NL^T @ T             (linear S-terms + alpha*V)
              + J^T @ W                   (diagonal row-validity mask)
              + band_SD^T @ cl            (-100*omega*d(s) diagonal)
       acc   += sum(Square(PSUM_R))       (one ACT activation per pair)
  * All inputs for a pair (2x V tile + 6 band matrices) ship as ONE DMA.
  * Row interior masking (s=0, s=4095, overlap rows) is via zeroed band
    columns; column masking (t=0, t=4095) and the two boundary losses are
    cheap O(N) host-side numpy corrections.
  * Host gathers per-core [128, 17] partial sums and assembles the scalar.

bf16 note: the loss is a 16.7M-element mean of squares, so per-element bf16
quantization noise averages out (measured ~1e-5 relative vs the fp32
reference; fp32/f32r variant of this kernel measured 7.6e-8).
"""

import math

import ml_dtypes
import numpy as np

import concourse.bacc as bacc
import concourse.bass as bass  # noqa: F401  (kept for users of this module)
import concourse.bass_utils as bass_utils
import concourse.mybir as mybir
import concourse.tile as tile

# ----------------------------------------------------------------------------
# Problem constants (must match reference.py)
# ----------------------------------------------------------------------------
N_S = 4096
N_T = 4096
R = 0.05
SIGMA = 0.2
K_STRIKE = 100.0
T_MAT = 1.0
S_MAX = 300.0
B_BARRIER = 100.0
ALPHA_STR = 30.0
CHI = 6.0
LAMBDA_PDE = 1.0
LAMBDA_BC = 10.0
LAMBDA_TC = 10.0
HUBER_DELTA = 0.01
ALPHA = 2.0 * R / SIGMA**2
TAU_MAX = 0.5 * SIGMA**2 * T_MAT
DU = 1.0 / (N_S - 1)
DT_NORM = TAU_MAX / (N_T - 1)
OMEGA = 2.0 * DT_NORM  # residual is computed on-device scaled by OMEGA

# ----------------------------------------------------------------------------
# Kernel geometry
# ----------------------------------------------------------------------------
N_CORES = 8
OWN_COLS = N_T // N_CORES  # 512 owned t-columns per core
SH_COLS = OWN_COLS + 2  # + halo col each side
TILE_STRIDE = 126  # 126 target rows per 128-row tile
N_TILES = 33  # ceil(4096 / 126)
N_PAIRS = (N_TILES + 1) // 2  # 17 (last pair has a zero-padded dummy slot)
PAD_ROWS = TILE_STRIDE * (N_TILES - 1) + 128  # 4160

# megatile layout (bf16 elements per partition-row, per tile-pair):
#   [T_a 514][T_b 514][bz_a 128][bn_a 128][bs_a 128][bz_b 128][bn_b 128][bs_b 128]
MEG_T0 = 0
MEG_T1 = SH_COLS
MEG_B0 = 2 * SH_COLS  # 1028
MEG_COLS = 2 * SH_COLS + 6 * 128  # 1796


def _solve_depressed_cubic(Q):
    p = CHI
    q = CHI * Q
    sp = math.sqrt(p)
    arg = abs(q) / (2.0 * p * sp / (3.0 * math.sqrt(3.0)))
    arg = max(1.0, arg)
    c = 2.0 * sp * math.cosh(math.acosh(arg) / 3.0)
    return -c if q >= 0 else c


C1 = _solve_depressed_cubic((B_BARRIER - 0.0) / ALPHA_STR)
C2 = _solve_depressed_cubic((B_BARRIER - S_MAX) / ALPHA_STR)


def _stretch_metrics(u):
    """CubicStretching.compute_metrics in float64. u may be any ndarray."""
    L = C2 * u + C1 * (1.0 - u)
    dL = C2 - C1
    S = B_BARRIER + ALPHA_STR * (L**3 / CHI + L)
    dS = ALPHA_STR * dL * (0.5 * L**2 + 1.0)
    d2S = ALPHA_STR * dL * dL * L
    return S, dS, d2S


def _row_coefs(s):
    """Per-row PDE coefficients, s = global S-row index (float64 array)."""
    u = s / (N_S - 1.0)
    S_phys, S_u, S_uu = _stretch_metrics(u)
    S_norm = S_phys / S_MAX
    S_u_n = S_u / S_MAX
    S_uu_n = S_uu / S_MAX
    A = 1.0 / (DU * DU * S_u_n * S_u_n)  # coef on D2 in V_SS
    Bc = -S_uu_n / (2.0 * DU * S_u_n**3)  # coef on D1 in V_SS
    d = S_norm * S_norm  # multiplies clipped V_SS
    g = OMEGA * ALPHA * S_norm / (2.0 * DU * S_u_n)  # coef on D1 (advection)
    return A, Bc, d, g


def _valid_targets(p):
    """Valid target partitions i (1..126) for tile p and their s-rows."""
    i = np.arange(1, 127)
    s_t = TILE_STRIDE * p + i - 1
    valid = (s_t >= 1) & (s_t <= N_S - 2)
    return i[valid], s_t[valid]


def _jvariant(p):
    """Which of the diagonal mask matrices the t-stencil matmul uses."""
    if p == 0:
        return 0
    if p == N_TILES - 1:
        return 2
    return 1


def _build_tables():
    """Per-tile band matrices [33, 128, 384] and J-mask matrices [4,128,128].

    bands[p][:, 0:128]   = band_Z (clip term, scaled so bounds are +-1)
    bands[p][:, 128:256] = band_NL (linear S terms)
    bands[p][:, 256:384] = band_SD (diagonal -100*omega*d)
    """
    bands = np.zeros((N_TILES, 128, 384), np.float64)
    for p in range(N_TILES):
        iv, s_t = _valid_targets(p)
        A, Bc, d, g = _row_coefs(s_t.astype(np.float64))
        bz = bands[p][:, 0:128]
        bn = bands[p][:, 128:256]
        bs = bands[p][:, 256:384]
        # band_Z': Z' = (A*D2 + B*D1)/100 so that clip bounds are +-1
        bz[iv + 1, iv] = (A + Bc) / 100.0
        bz[iv, iv] = -2.0 * A / 100.0
        bz[iv - 1, iv] = (A - Bc) / 100.0
        # band_NL: omega*(alpha*V - alpha*S_norm*V_S) -> taps -g, w*alpha, +g
        bn[iv + 1, iv] = -g
        bn[iv, iv] = OMEGA * ALPHA
        bn[iv - 1, iv] = g
        # subtract the clipped term scaled back by 100*omega*d(s)
        bs[iv, iv] = -100.0 * OMEGA * d
    jmask = np.zeros((4, 128, 128), np.float64)
    for var, p in {0: 0, 1: 1, 2: N_TILES - 1}.items():
        iv, _ = _valid_targets(p)
        jmask[var][iv, iv] = 1.0
    # variant 3 stays all-zero (dummy slot of the last pair)
    return bands, jmask


# megatile DMA grouping: first group small (compute starts early), later
# groups large (amortize the ~0.6us per-DMA issue cost on the Sync queue).
DMA_GROUPS = [(0, 1), (1, 3), (3, 5), (5, 9), (9, 13), (13, N_PAIRS)]


def _build_nc():
    dt_mm = mybir.dt.bfloat16
    f32 = mybir.dt.float32
    nc = bacc.Bacc("TRN2", target_bir_lowering=False, debug=False,
                   num_devices=N_CORES)
    meg_d = nc.dram_tensor("meg", [N_PAIRS, 128, MEG_COLS], dt_mm,
                           kind="ExternalInput").ap()
    jmask_d = nc.dram_tensor("jmask", [128, 4 * 128], dt_mm,
                             kind="ExternalInput").ap()
    acc_d = nc.dram_tensor("acc", [128, N_PAIRS], f32,
                           kind="ExternalOutput").ap()

    group_of = {}
    for gi, (a, b) in enumerate(DMA_GROUPS):
        for k in range(a, b):
            group_of[k] = gi
    max_group = max(b - a for a, b in DMA_GROUPS)

    with tile.TileContext(nc) as tc:
        with (
            tc.tile_pool(name="consts", bufs=1) as consts,
            tc.tile_pool(name="inp", bufs=2) as inp,
            tc.tile_pool(name="work", bufs=3) as workp,
            tc.tile_pool(name="psum", bufs=2, space="PSUM") as psump,
        ):
            jmask_t = consts.tile([128, 4 * 128], dt_mm)
            nc.sync.dma_start(out=jmask_t[:], in_=jmask_d)
            acc_t = consts.tile([128, N_PAIRS], f32)
            nc.vector.memset(acc_t[:], 0.0)

            mg_tiles = {}

            def stage_front(k):
                """DMA (on group boundary), Z-matmuls, clip, t-diff of pair k."""
                gi = group_of[k]
                a, b = DMA_GROUPS[gi]
                if k == a:
                    mg = inp.tile([128, max_group, MEG_COLS], dt_mm,
                                  tag="mg")
                    nc.sync.dma_start(
                        out=mg[:, 0:b - a, :],
                        in_=meg_d[a:b].rearrange("n p c -> p n c"))
                    mg_tiles[gi] = mg
                M = mg_tiles[gi][:, k - DMA_GROUPS[group_of[k]][0], :]

                psz = psump.tile([128, 2 * OWN_COLS], f32, tag="psz")
                for h in range(2):
                    b0 = MEG_B0 + h * 384
                    t0 = h * SH_COLS
                    nc.tensor.matmul(
                        psz[:, h * OWN_COLS:(h + 1) * OWN_COLS],
                        lhsT=M[:, b0:b0 + 128],
                        rhs=M[:, t0 + 1:t0 + 1 + OWN_COLS],
                        start=True, stop=True)

                cl = workp.tile([128, 2 * OWN_COLS], dt_mm, tag="cl")
                nc.vector.tensor_scalar(
                    cl[:], psz[:], -1.0, 1.0,
                    mybir.AluOpType.max, mybir.AluOpType.min)

                # t-stencil difference, both tiles at once; alternate the
                # engine so DVE (busy with clips) and GPSIMD share the load.
                W = workp.tile([128, 2, OWN_COLS], dt_mm, tag="W")
                tpair = M[:, 0:2 * SH_COLS].rearrange(
                    "p (a c) -> p a c", a=2)
                eng = nc.vector if (k % 2 == 0) else nc.gpsimd
                eng.tensor_sub(W[:], tpair[:, :, 2:2 + OWN_COLS],
                               tpair[:, :, 0:OWN_COLS])
                return M, cl, W

            def stage_back(k, ctx):
                """psr accumulation matmuls + squared-sum of pair k."""
                M, cl, W = ctx
                psr = psump.tile([128, 2 * OWN_COLS], f32, tag="psr")
                for h in range(2):
                    p = 2 * k + h
                    b0 = MEG_B0 + h * 384
                    t0 = h * SH_COLS
                    v = _jvariant(p) if p < N_TILES else 3
                    out_h = psr[:, h * OWN_COLS:(h + 1) * OWN_COLS]
                    nc.tensor.matmul(
                        out_h, lhsT=M[:, b0 + 128:b0 + 256],
                        rhs=M[:, t0 + 1:t0 + 1 + OWN_COLS],
                        start=True, stop=False)
                    nc.tensor.matmul(
                        out_h, lhsT=jmask_t[:, v * 128:(v + 1) * 128],
                        rhs=W[:, h, :],
                        start=False, stop=False)
                    nc.tensor.matmul(
                        out_h, lhsT=M[:, b0 + 256:b0 + 384],
                        rhs=cl[:, h * OWN_COLS:(h + 1) * OWN_COLS],
                        start=False, stop=True)

                sq = workp.tile([128, 2 * OWN_COLS], f32, tag="sq")
                nc.scalar.activation(
                    out=sq[:], in_=psr[:],
                    func=mybir.ActivationFunctionType.Square,
                    accum_out=acc_t[:, k:k + 1])

            # 2-stage software pipeline: pair k's psr-group matmuls are
            # emitted one iteration later, so every PE instruction's inputs
            # (cl, W of the previous pair) are ready and the PE streams.
            prev = None
            for k in range(N_PAIRS):
                ctx = stage_front(k)
                if prev is not None:
                    stage_back(k - 1, prev)
                prev = ctx
            stage_back(N_PAIRS - 1, prev)

            nc.sync.dma_start(out=acc_d, in_=acc_t[:])
    nc.compile()
    return nc


# ----------------------------------------------------------------------------
# Host-side exact helpers (float64)
# ----------------------------------------------------------------------------
def _host_residual_cols(V, cols):
    """residual[s, j] for s=1..N_S-2 at the given t-columns j (exact formula).

    V is the full [N_S, N_T] float array. Returns [N_S-2, len(cols)]."""
    V = V.astype(np.float64)
    out = []
    s = np.arange(1, N_S - 1)
    u = s / (N_S - 1.0)
    S_phys, S_u, S_uu = _stretch_metrics(u)
    S_norm = S_phys / S_MAX
    S_u_n = S_u / S_MAX
    for j in cols:
        jm = max(j - 1, 0)
        jp = min(j + 1, N_T - 1)
        v_c = V[1:-1, j]
        v_up = V[0:-2, j]
        v_dn = V[2:, j]
        D1 = v_dn - v_up
        D2 = v_dn - 2.0 * v_c + v_up
        V_u = D1 / (2.0 * DU)
        V_uu = D2 / (DU * DU)
        V_t = (V[1:-1, jp] - V[1:-1, jm]) / (2.0 * DT_NORM)
        V_S = V_u / S_u_n
        V_SS = (V_uu * S_u_n - V_u * (S_uu / S_MAX)) / S_u_n**3
        V_SS = np.clip(V_SS, -100.0, 100.0)
        res = V_t - S_norm**2 * V_SS - ALPHA * S_norm * V_S + ALPHA * v_c
        out.append(res)
    return np.stack(out, axis=1)


def _host_boundary_losses(V):
    V64 = V.astype(np.float64)
    t = np.linspace(0.0, 1.0, N_T)
    bc_target = 1.0 - K_STRIKE * np.exp(-R * (1.0 - t)) / S_MAX
    loss_smax = np.sum((V64[-1, :] - bc_target) ** 2) / N_T

    u = np.linspace(0.0, 1.0, N_S)
    x = 50.0 * (u - K_STRIKE / S_MAX)
    payoff = np.logaddexp(0.0, x) / 50.0  # softplus(x)/50
    diff = V64[:, -1] - payoff
    ad = np.abs(diff)
    hub = np.where(ad < HUBER_DELTA, 0.5 * diff * diff,
                   HUBER_DELTA * (ad - 0.5 * HUBER_DELTA))
    loss_t = np.sum(hub) / N_S
    return loss_smax, loss_t


def _make_in_maps(V):
    """V: [N_S, N_T] float32. Returns per-core input dicts."""
    bands, jmask = _build_tables()
    bands16 = bands.astype(ml_dtypes.bfloat16)
    # jmask ships as [128, 4*128] (one DMA): row k holds the 4 variants' k-th
    # source rows side by side.
    jmask16 = np.ascontiguousarray(
        jmask.transpose(1, 0, 2).reshape(128, 4 * 128)
    ).astype(ml_dtypes.bfloat16)
    # pad t with halo columns (edge), pad S rows for the tile grid.
    Vp = np.pad(V, ((0, 0), (1, 1)), mode="edge")  # [4096, 4098]
    vsh_full = np.empty((PAD_ROWS, N_T + 2), np.float32)
    vsh_full[1:N_S + 1] = Vp
    vsh_full[0] = Vp[0]
    vsh_full[N_S + 1:] = Vp[-1]
    vsh16 = vsh_full.astype(ml_dtypes.bfloat16)

    in_maps = []
    for k in range(N_CORES):
        c0 = OWN_COLS * k
        meg = np.zeros((N_PAIRS, 128, MEG_COLS), ml_dtypes.bfloat16)
        for pair in range(N_PAIRS):
            for h in range(2):
                p = 2 * pair + h
                if p >= N_TILES:
                    continue  # dummy slot stays zero
                r0 = TILE_STRIDE * p
                meg[pair, :, h * SH_COLS:(h + 1) * SH_COLS] = \
                    vsh16[r0:r0 + 128, c0:c0 + SH_COLS]
                meg[pair, :, MEG_B0 + h * 384:MEG_B0 + (h + 1) * 384] = \
                    bands16[p]
        in_maps.append({"meg": meg, "jmask": jmask16})
    return in_maps


TRACE = False
LAST_RESULTS = None


def _assemble(V, accs):
    """Combine device partial sums + host boundary terms into the loss."""
    device_r2 = float(sum(a.astype(np.float64).sum() for a in accs))
    # device_r2 = omega^2 * sum residual^2 over s in [1,4094], j in [0,4095]
    corr = _host_residual_cols(V, [0, N_T - 1])
    corr_sum = float((corr ** 2).sum())
    pde_sum = device_r2 / (OMEGA * OMEGA) - corr_sum
    n_int = (N_S - 2) * (N_T - 2)
    pde_loss = pde_sum / n_int
    loss_smax, loss_t = _host_boundary_losses(V)
    total = (LAMBDA_PDE * pde_loss + LAMBDA_BC * loss_smax
             + LAMBDA_TC * loss_t)
    return np.float32(total)


def kernel(V_norm):
    global LAST_RESULTS
    V = np.ascontiguousarray(np.asarray(V_norm, dtype=np.float32)[0, 0])
    in_maps = _make_in_maps(V)
    nc = _build_nc()
    res = bass_utils.run_bass_kernel_spmd(
        nc, in_maps, core_ids=list(range(N_CORES)), trace=TRACE)
    LAST_RESULTS = res
    accs = [r["acc"] for r in res.results]
    return _assemble(V, accs)


# revision 26
# speedup vs baseline: 1.4047x; 1.0004x over previous
"""Trainium2 Bass kernel for nn_BSLoss_13640816132730.

Computes the Black-Scholes PINN loss from reference.py:
  - pde_loss: mean squared PDE residual over the interior of a 4096x4096
    (S x t) grid, with 3-point finite-difference stencils along both axes
    and a clipped second-derivative term.
  - two small boundary losses (far-field BC row, terminal-condition col).

Strategy (8 NeuronCores, data-parallel over t):
  * Each core owns 512 t-columns (+1 halo column each side).
  * The S axis is processed in 33 overlapping 128-row tiles (stride 126), so
    every S-stencil is contained in one tile; tiles are handled in PAIRS so
    DMA / DVE / ACT fixed costs amortize over 1024-column operations.
  * Per tile the heavy lifting runs on the PE array as bf16 matmuls with
    banded "stencil" matrices (coefficients folded in host-side):
       PSUM_Z = band_Z^T @ T              (to-be-clipped V_SS term, scaled
                                           so the clip bounds are +-1)
       cl     = clip(PSUM_Z, -1, 1)       (one DVE tensor_scalar per pair)
       W      = T[:,+1] - T[:,-1]         (t-stencil; DVE/GPSIMD alternating)
       PSUM_R = band_NL^T @ T             (linear S-terms + alpha*V)
              + J^T @ W                   (diagonal row-validity mask)
              + band_SD^T @ cl            (-100*omega*d(s) diagonal)
       acc   += sum(Square(PSUM_R))       (one ACT activation per pair)
  * All inputs for a pair (2x V tile + 6 band matrices) ship as ONE DMA.
  * Row interior masking (s=0, s=4095, overlap rows) is via zeroed band
    columns; column masking (t=0, t=4095) and the two boundary losses are
    cheap O(N) host-side numpy corrections.
  * Host gathers per-core [128, 17] partial sums and assembles the scalar.

bf16 note: the loss is a 16.7M-element mean of squares, so per-element bf16
quantization noise averages out (measured ~1e-5 relative vs the fp32
reference; fp32/f32r variant of this kernel measured 7.6e-8).
"""

import math

import ml_dtypes
import numpy as np

import concourse.bacc as bacc
import concourse.bass as bass  # noqa: F401  (kept for users of this module)
import concourse.bass_utils as bass_utils
import concourse.mybir as mybir
import concourse.tile as tile

# ----------------------------------------------------------------------------
# Problem constants (must match reference.py)
# ----------------------------------------------------------------------------
N_S = 4096
N_T = 4096
R = 0.05
SIGMA = 0.2
K_STRIKE = 100.0
T_MAT = 1.0
S_MAX = 300.0
B_BARRIER = 100.0
ALPHA_STR = 30.0
CHI = 6.0
LAMBDA_PDE = 1.0
LAMBDA_BC = 10.0
LAMBDA_TC = 10.0
HUBER_DELTA = 0.01
ALPHA = 2.0 * R / SIGMA**2
TAU_MAX = 0.5 * SIGMA**2 * T_MAT
DU = 1.0 / (N_S - 1)
DT_NORM = TAU_MAX / (N_T - 1)
OMEGA = 2.0 * DT_NORM  # residual is computed on-device scaled by OMEGA

# ----------------------------------------------------------------------------
# Kernel geometry
# ----------------------------------------------------------------------------
N_CORES = 8
OWN_COLS = N_T // N_CORES  # 512 owned t-columns per core
SH_COLS = OWN_COLS + 2  # + halo col each side
TILE_STRIDE = 126  # 126 target rows per 128-row tile
N_TILES = 33  # ceil(4096 / 126)
N_PAIRS = (N_TILES + 1) // 2  # 17 (last pair has a zero-padded dummy slot)
PAD_ROWS = TILE_STRIDE * (N_TILES - 1) + 128  # 4160

# megatile layout (bf16 elements per partition-row, per tile-pair):
#   [T_a 514][T_b 514][bz_a 128][bn_a 128][bs_a 128][bz_b 128][bn_b 128][bs_b 128]
MEG_T0 = 0
MEG_T1 = SH_COLS
MEG_B0 = 2 * SH_COLS  # 1028
MEG_COLS = 2 * SH_COLS + 6 * 128  # 1796


def _solve_depressed_cubic(Q):
    p = CHI
    q = CHI * Q
    sp = math.sqrt(p)
    arg = abs(q) / (2.0 * p * sp / (3.0 * math.sqrt(3.0)))
    arg = max(1.0, arg)
    c = 2.0 * sp * math.cosh(math.acosh(arg) / 3.0)
    return -c if q >= 0 else c


C1 = _solve_depressed_cubic((B_BARRIER - 0.0) / ALPHA_STR)
C2 = _solve_depressed_cubic((B_BARRIER - S_MAX) / ALPHA_STR)


def _stretch_metrics(u):
    """CubicStretching.compute_metrics in float64. u may be any ndarray."""
    L = C2 * u + C1 * (1.0 - u)
    dL = C2 - C1
    S = B_BARRIER + ALPHA_STR * (L**3 / CHI + L)
    dS = ALPHA_STR * dL * (0.5 * L**2 + 1.0)
    d2S = ALPHA_STR * dL * dL * L
    return S, dS, d2S


def _row_coefs(s):
    """Per-row PDE coefficients, s = global S-row index (float64 array)."""
    u = s / (N_S - 1.0)
    S_phys, S_u, S_uu = _stretch_metrics(u)
    S_norm = S_phys / S_MAX
    S_u_n = S_u / S_MAX
    S_uu_n = S_uu / S_MAX
    A = 1.0 / (DU * DU * S_u_n * S_u_n)  # coef on D2 in V_SS
    Bc = -S_uu_n / (2.0 * DU * S_u_n**3)  # coef on D1 in V_SS
    d = S_norm * S_norm  # multiplies clipped V_SS
    g = OMEGA * ALPHA * S_norm / (2.0 * DU * S_u_n)  # coef on D1 (advection)
    return A, Bc, d, g


def _valid_targets(p):
    """Valid target partitions i (1..126) for tile p and their s-rows."""
    i = np.arange(1, 127)
    s_t = TILE_STRIDE * p + i - 1
    valid = (s_t >= 1) & (s_t <= N_S - 2)
    return i[valid], s_t[valid]


def _jvariant(p):
    """Which of the diagonal mask matrices the t-stencil matmul uses."""
    if p == 0:
        return 0
    if p == N_TILES - 1:
        return 2
    return 1


def _build_tables():
    """Per-tile band matrices [33, 128, 384] and J-mask matrices [4,128,128].

    bands[p][:, 0:128]   = band_Z (clip term, scaled so bounds are +-1)
    bands[p][:, 128:256] = band_NL (linear S terms)
    bands[p][:, 256:384] = band_SD (diagonal -100*omega*d)
    """
    bands = np.zeros((N_TILES, 128, 384), np.float64)
    for p in range(N_TILES):
        iv, s_t = _valid_targets(p)
        A, Bc, d, g = _row_coefs(s_t.astype(np.float64))
        bz = bands[p][:, 0:128]
        bn = bands[p][:, 128:256]
        bs = bands[p][:, 256:384]
        # band_Z': Z' = (A*D2 + B*D1)/100 so that clip bounds are +-1
        bz[iv + 1, iv] = (A + Bc) / 100.0
        bz[iv, iv] = -2.0 * A / 100.0
        bz[iv - 1, iv] = (A - Bc) / 100.0
        # band_NL: omega*(alpha*V - alpha*S_norm*V_S) -> taps -g, w*alpha, +g
        bn[iv + 1, iv] = -g
        bn[iv, iv] = OMEGA * ALPHA
        bn[iv - 1, iv] = g
        # subtract the clipped term scaled back by 100*omega*d(s)
        bs[iv, iv] = -100.0 * OMEGA * d
    jmask = np.zeros((4, 128, 128), np.float64)
    for var, p in {0: 0, 1: 1, 2: N_TILES - 1}.items():
        iv, _ = _valid_targets(p)
        jmask[var][iv, iv] = 1.0
    # variant 3 stays all-zero (dummy slot of the last pair)
    return bands, jmask


# megatile DMA grouping: first group small (compute starts early), later
# groups large (amortize the ~0.6us per-DMA issue cost on the Sync queue).
DMA_GROUPS = [(0, 1), (1, 3), (3, 5), (5, 9), (9, 13), (13, N_PAIRS)]


def _build_nc():
    dt_mm = mybir.dt.bfloat16
    f32 = mybir.dt.float32
    nc = bacc.Bacc("TRN2", target_bir_lowering=False, debug=False,
                   num_devices=N_CORES)
    meg_d = nc.dram_tensor("meg", [N_PAIRS, 128, MEG_COLS], dt_mm,
                           kind="ExternalInput").ap()
    jmask_d = nc.dram_tensor("jmask", [128, 4 * 128], dt_mm,
                             kind="ExternalInput").ap()
    acc_d = nc.dram_tensor("acc", [128, N_PAIRS], f32,
                           kind="ExternalOutput").ap()

    group_of = {}
    for gi, (a, b) in enumerate(DMA_GROUPS):
        for k in range(a, b):
            group_of[k] = gi
    max_group = max(b - a for a, b in DMA_GROUPS)

    with tile.TileContext(nc) as tc:
        with (
            tc.tile_pool(name="consts", bufs=1) as consts,
            tc.tile_pool(name="inp", bufs=2) as inp,
            tc.tile_pool(name="work", bufs=4) as workp,
            tc.tile_pool(name="psum", bufs=2, space="PSUM") as psump,
        ):
            jmask_t = consts.tile([128, 4 * 128], dt_mm)
            nc.sync.dma_start(out=jmask_t[:], in_=jmask_d)
            acc_t = consts.tile([128, N_PAIRS], f32)
            nc.vector.memset(acc_t[:], 0.0)

            mg_tiles = {}

            def stage_front(k):
                """DMA (on group boundary), Z-matmuls, clip, t-diff of pair k."""
                gi = group_of[k]
                a, b = DMA_GROUPS[gi]
                if k == a:
                    mg = inp.tile([128, max_group, MEG_COLS], dt_mm,
                                  tag="mg")
                    nc.sync.dma_start(
                        out=mg[:, 0:b - a, :],
                        in_=meg_d[a:b].rearrange("n p c -> p n c"))
                    mg_tiles[gi] = mg
                M = mg_tiles[gi][:, k - DMA_GROUPS[group_of[k]][0], :]

                psz = psump.tile([128, 2 * OWN_COLS], f32, tag="psz")
                for h in range(2):
                    b0 = MEG_B0 + h * 384
                    t0 = h * SH_COLS
                    nc.tensor.matmul(
                        psz[:, h * OWN_COLS:(h + 1) * OWN_COLS],
                        lhsT=M[:, b0:b0 + 128],
                        rhs=M[:, t0 + 1:t0 + 1 + OWN_COLS],
                        start=True, stop=True)

                cl = workp.tile([128, 2 * OWN_COLS], dt_mm, tag="cl")
                nc.vector.tensor_scalar(
                    cl[:], psz[:], -1.0, 1.0,
                    mybir.AluOpType.max, mybir.AluOpType.min)

                # t-stencil difference; one half on DVE, one on GPSIMD so
                # neither engine gets a lumpy 2x-size op on its queue.
                W = workp.tile([128, 2, OWN_COLS], dt_mm, tag="W")
                tpair = M[:, 0:2 * SH_COLS].rearrange(
                    "p (a c) -> p a c", a=2)
                nc.vector.tensor_sub(W[:, 0, :],
                                     tpair[:, 0, 2:2 + OWN_COLS],
                                     tpair[:, 0, 0:OWN_COLS])
                nc.gpsimd.tensor_sub(W[:, 1, :],
                                     tpair[:, 1, 2:2 + OWN_COLS],
                                     tpair[:, 1, 0:OWN_COLS])
                return M, cl, W

            def stage_back(k, ctx):
                """psr accumulation matmuls + squared-sum of pair k."""
                M, cl, W = ctx
                psr = psump.tile([128, 2 * OWN_COLS], f32, tag="psr")
                for h in range(2):
                    p = 2 * k + h
                    b0 = MEG_B0 + h * 384
                    t0 = h * SH_COLS
                    v = _jvariant(p) if p < N_TILES else 3
                    out_h = psr[:, h * OWN_COLS:(h + 1) * OWN_COLS]
                    nc.tensor.matmul(
                        out_h, lhsT=M[:, b0 + 128:b0 + 256],
                        rhs=M[:, t0 + 1:t0 + 1 + OWN_COLS],
                        start=True, stop=False)
                    nc.tensor.matmul(
                        out_h, lhsT=jmask_t[:, v * 128:(v + 1) * 128],
                        rhs=W[:, h, :],
                        start=False, stop=False)
                    nc.tensor.matmul(
                        out_h, lhsT=M[:, b0 + 256:b0 + 384],
                        rhs=cl[:, h * OWN_COLS:(h + 1) * OWN_COLS],
                        start=False, stop=True)

                sq = workp.tile([128, 2 * OWN_COLS], f32, tag="sq")
                nc.scalar.activation(
                    out=sq[:], in_=psr[:],
                    func=mybir.ActivationFunctionType.Square,
                    accum_out=acc_t[:, k:k + 1])

            # 2-stage software pipeline: pair k's psr-group matmuls are
            # emitted one iteration later, so every PE instruction's inputs
            # (cl, W of the previous pair) are ready and the PE streams.
            prev = None
            for k in range(N_PAIRS):
                ctx = stage_front(k)
                if prev is not None:
                    stage_back(k - 1, prev)
                prev = ctx
            stage_back(N_PAIRS - 1, prev)

            nc.sync.dma_start(out=acc_d, in_=acc_t[:])
    nc.compile()
    return nc


# ----------------------------------------------------------------------------
# Host-side exact helpers (float64)
# ----------------------------------------------------------------------------
def _host_residual_cols(V, cols):
    """residual[s, j] for s=1..N_S-2 at the given t-columns j (exact formula).

    V is the full [N_S, N_T] float array. Returns [N_S-2, len(cols)]."""
    V = V.astype(np.float64)
    out = []
    s = np.arange(1, N_S - 1)
    u = s / (N_S - 1.0)
    S_phys, S_u, S_uu = _stretch_metrics(u)
    S_norm = S_phys / S_MAX
    S_u_n = S_u / S_MAX
    for j in cols:
        jm = max(j - 1, 0)
        jp = min(j + 1, N_T - 1)
        v_c = V[1:-1, j]
        v_up = V[0:-2, j]
        v_dn = V[2:, j]
        D1 = v_dn - v_up
        D2 = v_dn - 2.0 * v_c + v_up
        V_u = D1 / (2.0 * DU)
        V_uu = D2 / (DU * DU)
        V_t = (V[1:-1, jp] - V[1:-1, jm]) / (2.0 * DT_NORM)
        V_S = V_u / S_u_n
        V_SS = (V_uu * S_u_n - V_u * (S_uu / S_MAX)) / S_u_n**3
        V_SS = np.clip(V_SS, -100.0, 100.0)
        res = V_t - S_norm**2 * V_SS - ALPHA * S_norm * V_S + ALPHA * v_c
        out.append(res)
    return np.stack(out, axis=1)


def _host_boundary_losses(V):
    V64 = V.astype(np.float64)
    t = np.linspace(0.0, 1.0, N_T)
    bc_target = 1.0 - K_STRIKE * np.exp(-R * (1.0 - t)) / S_MAX
    loss_smax = np.sum((V64[-1, :] - bc_target) ** 2) / N_T

    u = np.linspace(0.0, 1.0, N_S)
    x = 50.0 * (u - K_STRIKE / S_MAX)
    payoff = np.logaddexp(0.0, x) / 50.0  # softplus(x)/50
    diff = V64[:, -1] - payoff
    ad = np.abs(diff)
    hub = np.where(ad < HUBER_DELTA, 0.5 * diff * diff,
                   HUBER_DELTA * (ad - 0.5 * HUBER_DELTA))
    loss_t = np.sum(hub) / N_S
    return loss_smax, loss_t


def _make_in_maps(V):
    """V: [N_S, N_T] float32. Returns per-core input dicts."""
    bands, jmask = _build_tables()
    bands16 = bands.astype(ml_dtypes.bfloat16)
    # jmask ships as [128, 4*128] (one DMA): row k holds the 4 variants' k-th
    # source rows side by side.
    jmask16 = np.ascontiguousarray(
        jmask.transpose(1, 0, 2).reshape(128, 4 * 128)
    ).astype(ml_dtypes.bfloat16)
    # pad t with halo columns (edge), pad S rows for the tile grid.
    Vp = np.pad(V, ((0, 0), (1, 1)), mode="edge")  # [4096, 4098]
    vsh_full = np.empty((PAD_ROWS, N_T + 2), np.float32)
    vsh_full[1:N_S + 1] = Vp
    vsh_full[0] = Vp[0]
    vsh_full[N_S + 1:] = Vp[-1]
    vsh16 = vsh_full.astype(ml_dtypes.bfloat16)

    in_maps = []
    for k in range(N_CORES):
        c0 = OWN_COLS * k
        meg = np.zeros((N_PAIRS, 128, MEG_COLS), ml_dtypes.bfloat16)
        for pair in range(N_PAIRS):
            for h in range(2):
                p = 2 * pair + h
                if p >= N_TILES:
                    continue  # dummy slot stays zero
                r0 = TILE_STRIDE * p
                meg[pair, :, h * SH_COLS:(h + 1) * SH_COLS] = \
                    vsh16[r0:r0 + 128, c0:c0 + SH_COLS]
                meg[pair, :, MEG_B0 + h * 384:MEG_B0 + (h + 1) * 384] = \
                    bands16[p]
        in_maps.append({"meg": meg, "jmask": jmask16})
    return in_maps


TRACE = False
LAST_RESULTS = None


def _assemble(V, accs):
    """Combine device partial sums + host boundary terms into the loss."""
    device_r2 = float(sum(a.astype(np.float64).sum() for a in accs))
    # device_r2 = omega^2 * sum residual^2 over s in [1,4094], j in [0,4095]
    corr = _host_residual_cols(V, [0, N_T - 1])
    corr_sum = float((corr ** 2).sum())
    pde_sum = device_r2 / (OMEGA * OMEGA) - corr_sum
    n_int = (N_S - 2) * (N_T - 2)
    pde_loss = pde_sum / n_int
    loss_smax, loss_t = _host_boundary_losses(V)
    total = (LAMBDA_PDE * pde_loss + LAMBDA_BC * loss_smax
             + LAMBDA_TC * loss_t)
    return np.float32(total)


def kernel(V_norm):
    global LAST_RESULTS
    V = np.ascontiguousarray(np.asarray(V_norm, dtype=np.float32)[0, 0])
    in_maps = _make_in_maps(V)
    nc = _build_nc()
    res = bass_utils.run_bass_kernel_spmd(
        nc, in_maps, core_ids=list(range(N_CORES)), trace=TRACE)
    LAST_RESULTS = res
    accs = [r["acc"] for r in res.results]
    return _assemble(V, accs)
